# revision 5
# baseline (speedup 1.0000x reference)
"""Trainium2 Bass kernel for GQA attention layer (RoPE + causal + GQA 32q/8kv).

Self-contained: hardcodes shapes from the problem spec.
  hidden_states [2, 2048, 4096] f32, positions [2, 2048] i32,
  Wq [4096, 4096], Wk [1024, 4096], Wv [1024, 4096], Wo [4096, 4096]  (all f32)
Sharding: tensor-parallel over heads across 8 cores. Core c gets kv head c and
q heads 4c..4c+3. Each core computes its partial Wo output; host sums partials.

Per-core schedule (per batch, chunk = 512 tokens = one q-chunk):
  proj(0), attn(0), proj(1), wo(0), attn(1), ..., proj(3), wo(2), attn(3), wo(3)
Attention keeps scores in [k, q] orientation; ctx^T is computed directly with
V as the stationary operand (N=512 matmuls), the softmax denominator comes from
packed M=1 ones-matmuls, and normalization is a reciprocal + DMA partition
broadcast + one multiply per head.
"""

import math
import os
import sys
import types

import numpy as np
import ml_dtypes

BF16NP = ml_dtypes.bfloat16

# ---- problem constants (hardcoded per spec) ----
P = 128
B = 2
S = 2048            # tokens per batch
HID = 4096
NH, NKV, HD = 32, 8, 128
NCORES = 8
HPC = NH // NCORES  # q heads per core (4)
T = B * S
SCALE = 1.0 / math.sqrt(HD)
ROPE_BASE = 10000.0

QC = 512            # q-chunk == proj chunk (tokens)
NQC = S // QC       # 4
NKK = HID // P      # 32 contraction tiles
NKT = S // P        # 16 k tiles per batch
OCW = 512           # Wo output column chunk
HOC = HID // OCW    # 8
MASK_OFF = [0, 128, 384, 768]   # per-diagonal mask tile offsets, widths (d+1)*128
MASK_W = 1280

LAST = {}           # exec_time_ns etc from the most recent run


def _install_ntff_hook():
    """Register the axon NTFF profiling hook (image's antenv lacks axon_hooks)."""
    if "antenv.axon_hooks" in sys.modules:
        return
    try:
        import antenv
        mod = types.ModuleType("antenv.axon_hooks")
        _box = [None]
        mod.set_axon_ntff_profile_hook = lambda h: _box.__setitem__(0, h)
        mod.get_axon_ntff_profile_hook = lambda: _box[0]
        sys.modules["antenv.axon_hooks"] = mod
        antenv.axon_hooks = mod
        from trn_agent_boot.trn_boot import _ntff_profile_via_ctypes
        mod.set_axon_ntff_profile_hook(
            _ntff_profile_via_ctypes("/opt/axon/libaxon_pjrt.so")
        )
    except Exception:
        pass


def build_graph():
    import concourse.bacc as bacc
    import concourse.mybir as mybir
    import concourse.tile as tile
    from contextlib import ExitStack

    BF = mybir.dt.bfloat16
    F32 = mybir.dt.float32
    Exp = mybir.ActivationFunctionType.Exp

    NKH = NKK // 2  # x half-tiles

    nc = bacc.Bacc(None)
    xT_h = nc.declare_dram_parameter("xT", [HID, T], BF, isOutput=False)
    wq_h = nc.declare_dram_parameter("wqT", [HID, HPC * HD], BF, isOutput=False)
    wk_h = nc.declare_dram_parameter("wkT", [HID, HD], BF, isOutput=False)
    wv_h = nc.declare_dram_parameter("wvT", [HID, HD], BF, isOutput=False)
    wo_h = nc.declare_dram_parameter("woT", [HPC * HD, HID], BF, isOutput=False)
    cos_h = nc.declare_dram_parameter("cos2", [P, T], BF, isOutput=False)
    sin_h = nc.declare_dram_parameter("sin2", [P, T], BF, isOutput=False)
    msk_h = nc.declare_dram_parameter("masks", [P, MASK_W], BF, isOutput=False)
    out_h = nc.declare_dram_parameter("out", [T, HID], BF, isOutput=True)

    xT_r = xT_h[:, :].rearrange("(ko ki) s -> ki ko s", ki=P)
    wq_r = wq_h[:, :].rearrange("(ko ki) d -> ki ko d", ki=P)
    wk_r = wk_h[:, :].rearrange("(ko ki) d -> ki ko d", ki=P)
    wv_r = wv_h[:, :].rearrange("(ko ki) d -> ki ko d", ki=P)
    wo_r = wo_h[:, :].rearrange("(oo oi) h -> oi oo h", oi=P)

    with tile.TileContext(nc) as tc, ExitStack() as ctx:
        wpool = ctx.enter_context(tc.tile_pool(name="wpool", bufs=1))
        xpool = ctx.enter_context(tc.tile_pool(name="xpool", bufs=3))
        cspool = ctx.enter_context(tc.tile_pool(name="cspool", bufs=2))
        qkvpool = ctx.enter_context(tc.tile_pool(name="qkvpool", bufs=1))
        vtpool = ctx.enter_context(tc.tile_pool(name="vtpool", bufs=2))
        rpool = ctx.enter_context(tc.tile_pool(name="rpool", bufs=2))
        pbpool = ctx.enter_context(tc.tile_pool(name="pbpool", bufs=3))
        rdpool = ctx.enter_context(tc.tile_pool(name="rdpool", bufs=2))
        rbpool = ctx.enter_context(tc.tile_pool(name="rbpool", bufs=2))
        obpool = ctx.enter_context(tc.tile_pool(name="obpool", bufs=2))

        drpool = ctx.enter_context(tc.tile_pool(name="drpool", bufs=2, space="DRAM"))
        big = ctx.enter_context(tc.tile_pool(name="big", bufs=2, space="PSUM"))
        ctxp = ctx.enter_context(tc.tile_pool(name="ctxp", bufs=1, space="PSUM"))
        denp = ctx.enter_context(tc.tile_pool(name="denp", bufs=1, space="PSUM"))

        # --- persistent weights / tables ---
        wq_sb = wpool.tile([P, NKK, HPC * HD], BF)
        for wi in range(4):
            lo, hi = wi * NKK // 4, (wi + 1) * NKK // 4
            nc.sync.dma_start(out=wq_sb[:, lo:hi, :], in_=wq_r[:, lo:hi, :])
        wk_sb = wpool.tile([P, NKK, HD], BF)
        nc.sync.dma_start(out=wk_sb, in_=wk_r)
        wv_sb = wpool.tile([P, NKK, HD], BF)
        nc.sync.dma_start(out=wv_sb, in_=wv_r)
        wo_sb = wpool.tile([P, HPC, HID], BF)
        msk_sb = wpool.tile([P, MASK_W], BF)
        nc.sync.dma_start(out=msk_sb, in_=msk_h[:, :])
        ones_sb = wpool.tile([P, 1], BF)
        nc.vector.memset(ones_sb, 1.0)

        def rope(ps, dst, cs, sn):
            """Neox RoPE on [128 d, n] tile: rows 0:64 = first half of head dim."""
            qf = rpool.tile([P, QC], BF, tag="qf")
            nc.any.tensor_copy(out=qf, in_=ps)
            qs = rpool.tile([P, QC], BF, tag="qs")
            nc.gpsimd.dma_start(out=qs[0:64, :], in_=qf[64:128, :])
            nc.gpsimd.dma_start(out=qs[64:128, :], in_=qf[0:64, :])
            nc.any.tensor_mul(out=qf, in0=qf, in1=cs)
            nc.any.tensor_mul(out=qs, in0=qs, in1=sn)
            nc.any.tensor_add(out=dst, in0=qf, in1=qs)

        def proj_chunk(b, t, qT2, kT, v):
            c0 = b * S + t * QC
            c1 = c0 + QC
            xta = xpool.tile([P, NKH, QC], BF, tag="x")
            nc.sync.dma_start(out=xta, in_=xT_r[:, 0:NKH, c0:c1])
            xtb = xpool.tile([P, NKH, QC], BF, tag="x")
            nc.sync.dma_start(out=xtb, in_=xT_r[:, NKH:NKK, c0:c1])
            cs = cspool.tile([P, QC], BF, tag="cos")
            nc.sync.dma_start(out=cs, in_=cos_h[:, c0:c1])
            sn = cspool.tile([P, QC], BF, tag="sin")
            nc.sync.dma_start(out=sn, in_=sin_h[:, c0:c1])

            def xt(kk):
                return xta[:, kk, :] if kk < NKH else xtb[:, kk - NKH, :]

            for g in range(HPC):
                ps = big.tile([P, 2, QC], F32, tag="big")
                for kk in range(NKK):
                    nc.tensor.matmul(
                        ps[:, 0, :],
                        lhsT=wq_sb[:, kk, g * HD:(g + 1) * HD],
                        rhs=xt(kk),
                        start=(kk == 0),
                        stop=(kk == NKK - 1),
                    )
                rope(ps[:, 0, :], qT2[:, t, g // 2, g % 2, :], cs, sn)
            ps = big.tile([P, 2, QC], F32, tag="big")
            for kk in range(NKK):
                nc.tensor.matmul(
                    ps[:, 0, :], lhsT=wk_sb[:, kk, :], rhs=xt(kk),
                    start=(kk == 0), stop=(kk == NKK - 1),
                )
            rope(ps[:, 0, :], kT[:, t * QC:(t + 1) * QC], cs, sn)
            # V in vT orientation (N=512 matmuls), then DMA-transpose to [s, d]
            pv = big.tile([P, 2, QC], F32, tag="big")
            for kk in range(NKK):
                nc.tensor.matmul(
                    pv[:, 0, :], lhsT=wv_sb[:, kk, :], rhs=xt(kk),
                    start=(kk == 0), stop=(kk == NKK - 1),
                )
            vt = vtpool.tile([P, QC], BF, tag="vt")
            nc.any.tensor_copy(out=vt, in_=pv[:, 0, :])
            for ss in range(QC // P):
                nc.scalar.dma_start(
                    out=v[:, t * (QC // P) + ss, :],
                    in_=vt[:, ss * P:(ss + 1) * P],
                    transpose=True,
                )

        def attn_chunk(b, qc, qT2, kT, v, ctxT):
            nkt = 4 * (qc + 1)
            for p2 in range(2):
                cx0 = ctxp.tile([P, QC], F32, tag="c0", name="cx0")
                cx1 = ctxp.tile([P, QC], F32, tag="c1", name="cx1")
                cx = [cx0, cx1]
                den = denp.tile([P, QC], F32, tag="d")
                for kt in range(nkt):
                    sc = big.tile([P, 2, QC], F32, tag="big")
                    for h in range(2):
                        nc.tensor.matmul(
                            sc[:, h, :],
                            lhsT=kT[:, kt * P:(kt + 1) * P],
                            rhs=qT2[:, qc, p2, h, :],
                            start=True, stop=True,
                        )
                    d = kt - 4 * qc
                    if d >= 0:
                        w = (d + 1) * P
                        off = MASK_OFF[d]
                        nc.vector.tensor_add(
                            out=sc[:, :, 0:w],
                            in0=sc[:, :, 0:w],
                            in1=msk_sb[:, off:off + w].unsqueeze(1)
                                 .to_broadcast([P, 2, w]),
                        )
                    pb = pbpool.tile([P, 2, QC], BF, tag="pb")
                    nc.scalar.activation(out=pb, in_=sc, func=Exp, scale=SCALE)
                    first, last = kt == 0, kt == nkt - 1
                    for h in range(2):
                        nc.tensor.matmul(
                            cx[h], lhsT=v[:, kt, :], rhs=pb[:, h, :],
                            start=first, stop=last,
                        )
                        nc.tensor.matmul(
                            den[32 * h:32 * h + 1, :],
                            lhsT=ones_sb[:, 0:1], rhs=pb[:, h, :],
                            start=first, stop=last,
                            skip_group_check=True,
                        )
                rden = rdpool.tile([P, QC], F32, tag="rd")
                for h in range(2):
                    nc.vector.reciprocal(
                        out=rden[32 * h:32 * h + 1, :],
                        in_=den[32 * h:32 * h + 1, :],
                    )
                for h in range(2):
                    dr = drpool.tile([1, QC], F32, tag="dr", name="dr")
                    nc.sync.dma_start(out=dr, in_=rden[32 * h:32 * h + 1, :])
                    rb = rbpool.tile([P, QC], F32, tag="rb")
                    nc.sync.dma_start(
                        out=rb, in_=dr[0:1, :].to_broadcast([P, QC])
                    )
                    nc.vector.tensor_mul(
                        out=ctxT[:, 2 * p2 + h, qc * QC:(qc + 1) * QC],
                        in0=cx[h], in1=rb,
                    )

        def wo_block(b, qc, ctxT):
            for hc in range(HOC):
                for sg in range(2):
                    ob = obpool.tile([P, 2, OCW], BF, tag="ob")
                    for si2 in range(2):
                        si = sg * 2 + si2
                        po = big.tile([P, 2, QC], F32, tag="big")
                        for ot in range(HPC):
                            q0 = qc * QC + si * P
                            nc.tensor.matmul(
                                po[:, 0, :],
                                lhsT=ctxT[:, ot, q0:q0 + P],
                                rhs=wo_sb[:, ot, hc * OCW:(hc + 1) * OCW],
                                start=(ot == 0), stop=(ot == HPC - 1),
                            )
                        if (si2 + hc) % 2 == 0:
                            nc.vector.tensor_copy(out=ob[:, si2, :], in_=po[:, 0, :])
                        else:
                            nc.scalar.copy(out=ob[:, si2, :], in_=po[:, 0, :])
                    r0 = b * S + qc * QC + sg * 2 * P
                    orows = out_h[r0:r0 + 2 * P, hc * OCW:(hc + 1) * OCW]
                    nc.sync.dma_start(
                        out=orows.rearrange("(si p) h -> p si h", p=P), in_=ob
                    )

        for b in range(B):
            qT2 = qkvpool.tile([P, NQC, 2, 2, QC], BF, tag="qT2")
            kT = qkvpool.tile([P, S], BF, tag="kT")
            v = qkvpool.tile([P, NKT, P], BF, tag="v")
            ctxT = qkvpool.tile([P, HPC, S], BF, tag="ctxT")
            for t in range(NQC):
                proj_chunk(b, t, qT2, kT, v)
                if b == 0 and t == 0:
                    for wi in range(4):
                        lo, hi = wi * HID // 4, (wi + 1) * HID // 4
                        nc.sync.dma_start(
                            out=wo_sb[:, :, lo:hi], in_=wo_r[:, :, lo:hi]
                        )
                if t >= 1:
                    wo_block(b, t - 1, ctxT)
                attn_chunk(b, t, qT2, kT, v, ctxT)
            wo_block(b, NQC - 1, ctxT)

    nc.compile()
    return nc


_CACHE = {}


def _get_graph():
    if "nc" not in _CACHE:
        _CACHE["nc"] = build_graph()
    return _CACHE["nc"]


def _host_prep(hidden_states, positions, Wq, Wk, Wv, Wo):
    """Transpose/cast/slice inputs per core. Returns list of 8 input dicts."""
    x2 = np.ascontiguousarray(hidden_states.reshape(T, HID).T).astype(BF16NP)

    pos = positions.astype(np.float32)                      # [B, S]
    half = HD // 2
    inv_freq = 1.0 / (ROPE_BASE ** (np.arange(half, dtype=np.float32) / half))
    ang = pos[:, :, None] * inv_freq[None, None, :]         # [B, S, 64]
    cos = np.cos(ang)
    sin = np.sin(ang)
    cosT = np.concatenate([cos[b].T for b in range(B)], axis=1)   # [64, T]
    sinT = np.concatenate([sin[b].T for b in range(B)], axis=1)
    cos2 = np.concatenate([cosT, cosT], axis=0).astype(BF16NP)    # [128, T]
    sin2 = np.concatenate([-sinT, sinT], axis=0).astype(BF16NP)

    # causal mask tiles for diagonal blocks d=0..3: [128, (d+1)*128] each,
    # mask[k, off_d + j] = 0 if k <= j - d*128 else -1e30
    k = np.arange(P)[:, None]
    masks = np.zeros((P, MASK_W), np.float32)
    for d in range(4):
        w = (d + 1) * P
        j = np.arange(w)[None, :]
        masks[:, MASK_OFF[d]:MASK_OFF[d] + w] = np.where(
            k <= j - d * P, 0.0, -1e30
        )
    masks = masks.astype(BF16NP)

    in_maps = []
    for c in range(NCORES):
        qs = slice(c * HPC * HD, (c + 1) * HPC * HD)
        ks = slice(c * HD, (c + 1) * HD)
        in_maps.append({
            "xT": x2,
            "wqT": np.ascontiguousarray(Wq[qs, :].T).astype(BF16NP),
            "wkT": np.ascontiguousarray(Wk[ks, :].T).astype(BF16NP),
            "wvT": np.ascontiguousarray(Wv[ks, :].T).astype(BF16NP),
            "woT": np.ascontiguousarray(Wo[:, qs].T).astype(BF16NP),
            "cos2": cos2,
            "sin2": sin2,
            "masks": masks,
        })
    return in_maps


def kernel(hidden_states, positions, Wq, Wk, Wv, Wo):
    from concourse.bass_utils import run_bass_kernel_spmd

    trace = bool(os.environ.get("CLAUDE_KERNEL_TRACE"))
    if trace:
        _install_ntff_hook()

    nc = _get_graph()
    in_maps = _host_prep(
        np.asarray(hidden_states), np.asarray(positions),
        np.asarray(Wq), np.asarray(Wk), np.asarray(Wv), np.asarray(Wo),
    )
    res = run_bass_kernel_spmd(
        nc, in_maps, core_ids=list(range(NCORES)), trace=trace,
    )
    LAST["exec_time_ns"] = res.exec_time_ns
    LAST["profile_json"] = res.profile_json
    if res.instructions_and_trace is not None:
        LAST["trace_path"] = res.instructions_and_trace[1]

    acc = np.zeros((T, HID), np.float32)
    for c in range(NCORES):
        acc += res.results[c]["out"].astype(np.float32)
    return acc.reshape(B, S, HID)
